# revision 23
# baseline (speedup 1.0000x reference)
"""Trainium2 Bass kernel for a belief-transformer block (sparse attention).

Computation (per batch b):
    h   = LayerNorm(x[b]) * g1
    qkv = h @ w_qkv ; q,k,v = split(qkv)
    s   = q @ k^T / sqrt(D), keys j >= L_b masked
    y   = softmax(s) @ v
    y   = LayerNorm(y) * g2
    out = gelu(y @ w_fc) @ w_proj

Key optimizations over a dense data-parallel implementation:

* Sparsity: keys >= L_b contribute nothing (masked to -inf before softmax),
  so scores / attn@v / k,v-projection are computed only for the first
  ceil(L_b/128) key chunks. The SPMD program is JIT-specialized to per-slot
  chunk budgets: batches are sorted by chunk count and dealt round-robin so
  slot i's budget is the max chunk count of rank group [8i, 8i+8) -- the
  provably minimal total budget for an identical-program 8-core layout.
* M-trick: scores = (h @ M) @ h^T with M = (g1*wq/sqrt(D)) @ (g1*wk)^T folded
  on the host -- the entire K projection disappears.
* Feature-major attention output Y^T = V^T @ P^T: no transposes between
  attention and the MLP. LN2 statistics come out as [1, 512] rows: the mean
  via a vmsum column in the same 2-row matmul that computes the softmax
  denominator row r (with sqrt(eps) folded so eps*r^2 = row^2), sum-of-squares
  via a ones-column matmul over YSQ. rstd = Exp(-0.5*Ln(e)) on the ACT engine
  keeps everything inside the ln/exp activation table (no table switches).
  LN2 normalization is folded into the fc matmul: YT is pre-scaled by the
  rstd row and the mean term becomes a rank-1 correction row ([-colsum(wf)]
  stationary x (mu*rstd) moving) appended to the accumulation group.
* Attention weights (PT), masked values (VM) and the stats stationary (RS)
  are bf16: full-rate matmuls, half the SBUF traffic; error ~2.7e-3 vs the
  2e-2 gate.
* Engine balancing: PSUM->SBUF copybacks on the otherwise idle GpSimd/Pool
  engine, LN applies + row math on DVE, exp/gelu/squares/V-masking on ACT
  (Copy/Square live in every activation table), PE does only matmuls and
  128-wide f32r transposes (1.5 cyc/row). PE warm-up bursts at startup keep
  the p-state clock ramped.
"""

import os
import sys

import numpy as np


def _ensure_concourse():
    try:
        import concourse  # noqa: F401
        return
    except ImportError:
        pass
    for p in ("/root/.axon_site/_ro/trn_rl_repo", "/opt/trn_rl_repo"):
        if os.path.isdir(p) and p not in sys.path:
            sys.path.insert(0, p)
    import concourse  # noqa: F401


_ensure_concourse()

import concourse.tile as tile  # noqa: E402
from concourse import bacc, mybir  # noqa: E402
from concourse.bass_utils import run_bass_kernel_spmd  # noqa: E402
from concourse.masks import make_identity  # noqa: E402

B, N, D = 32, 1024, 512
NCORES = 8
G = B // NCORES  # batch slots per core
P = 128
NT = N // P      # token chunks (8)
DC = D // P      # feature chunks (4)
W_ = 512         # query window (2 windows per batch)
EPS = 1e-5
SQEPS = float(np.sqrt(np.float32(EPS)))

F32 = mybir.dt.float32
F32R = mybir.dt.float32r
BF16 = mybir.dt.bfloat16
ALU = mybir.AluOpType
ACT = mybir.ActivationFunctionType


def _body(ctx, tc, x, msk, wdram, out, warm, ms):
    nc = tc.nc

    singles = ctx.enter_context(tc.tile_pool(name="singles", bufs=1))
    xpool = ctx.enter_context(tc.tile_pool(name="xpool", bufs=2))
    htpool = ctx.enter_context(tc.tile_pool(name="htpool", bufs=2))
    qmpool = ctx.enter_context(tc.tile_pool(name="qmpool", bufs=1))
    vmpool = ctx.enter_context(tc.tile_pool(name="vmpool", bufs=1))
    ptpool = ctx.enter_context(tc.tile_pool(name="ptpool", bufs=2))
    ytpool = ctx.enter_context(tc.tile_pool(name="ytpool", bufs=2))
    ysqpool = ctx.enter_context(tc.tile_pool(name="ysqpool", bufs=1))
    ztpool = ctx.enter_context(tc.tile_pool(name="ztpool", bufs=1))
    rows = ctx.enter_context(tc.tile_pool(name="rows", bufs=1))
    zpool = ctx.enter_context(tc.tile_pool(name="zpool", bufs=2))
    rfull = ctx.enter_context(tc.tile_pool(name="rfull", bufs=2))
    stats = ctx.enter_context(tc.tile_pool(name="stats", bufs=2))
    outp = ctx.enter_context(tc.tile_pool(name="outp", bufs=2))
    ps_mm = ctx.enter_context(tc.tile_pool(name="ps_mm", bufs=4, space="PSUM"))
    ps_t = ctx.enter_context(tc.tile_pool(name="ps_t", bufs=2, space="PSUM"))
    ps_r = ctx.enter_context(tc.tile_pool(name="ps_r", bufs=1, space="PSUM"))

    # Replicated weights, feature-chunked [P, DC, D]; f32r for the PE.
    W = {}

    def load_w(name):
        t = singles.tile([P, DC, D], F32R, tag=name, name=name)
        nc.sync.dma_start(
            t[:], wdram[name].rearrange("(c p) e -> p c e", p=P).bitcast(F32R)
        )
        W[name] = t

    ident = singles.tile([P, P], F32R, tag="ident", name="ident")
    nc.sync.dma_start(ident[:], wdram["identr"].bitcast(F32R))
    onesD = singles.tile([P, 1], F32R, tag="onesD", name="onesD")
    nc.sync.dma_start(onesD[:], wdram["onesd"].bitcast(F32R))
    wfsp = singles.tile([P, DC], F32, tag="wfsp", name="wfsp")
    nc.sync.dma_start(wfsp[:], wdram["wfsp"])
    pm1su_t = singles.tile([2, 1], F32R, tag="pm1su", name="pm1su_t")
    nc.sync.dma_start(pm1su_t[:], wdram["pm1su"].bitcast(F32R))
    pm1su = pm1su_t[:]

    S = [dict() for _ in range(G)]  # per-slot live tiles

    def emit_xdma(b, split=False):
        s = S[b]
        xb = x[b].rearrange("(t p) d -> p t d", p=P)
        mb = msk[b].rearrange("(t p) -> p t", p=P)
        X = xpool.tile([P, NT, D], F32, tag="X", name="X")
        s["X"] = X
        if split:
            for pc in range(4):
                nc.sync.dma_start(
                    X[:, 2 * pc:2 * pc + 2, :].bitcast(F32R),
                    xb[:, 2 * pc:2 * pc + 2, :].bitcast(F32R),
                )
        else:
            nc.sync.dma_start(X[:].bitcast(F32R), xb.bitcast(F32R))
        s["mask_f"] = xpool.tile([P, NT], F32, tag="mask_f", name="mask_f")
        nc.sync.dma_start(s["mask_f"][:], mb)

    def emit_ln1_stats(b, t0=0, t1=NT):
        """LN1 stats for X chunks [t0, t1): bn stats + sd = Sqrt(var+eps) on
        ACT (one [P, n] op -- emit next to the other Sqrt users so they share
        the sqrt-table period)."""
        X = S[b]["X"]
        n = t1 - t0
        mvs = stats.tile([P, NT, 2], F32, tag="bnag", name="mvs", bufs=3)
        for t in range(t0, t1):
            st = stats.tile([P, 6], F32, tag="bnst")
            nc.vector.bn_stats(st[:], X[:, t, :])
            nc.vector.bn_aggr(mvs[:, t, :], st[:])
        e1 = stats.tile([P, NT], F32, tag="ln1e", name="e1", bufs=3)
        nc.vector.tensor_scalar(
            e1[:, 0:n], mvs[:, t0:t1, 1], EPS, None, op0=ALU.add
        )
        return mvs, e1

    def emit_ln1_finish(b, mvs, e1, t0=0, t1=NT, gate_ap=None):
        """Sqrt (ACT, cluster with other Sqrt users) + fast reciprocal, then
        the normalize applies on DVE."""
        X = S[b]["X"]
        n = t1 - t0
        if gate_ap is not None:
            gate(e1[0:1, 0:1], gate_ap)
        sd = stats.tile([P, NT], F32, tag="ln1sd", name="sd", bufs=3)
        nc.scalar.activation(sd[:, 0:n], e1[:, 0:n], ACT.Sqrt)
        rstd1x = stats.tile([P, NT], F32, tag="rstd1x", name="rstd1x", bufs=3)
        nc.vector.reciprocal_approx_fast(rstd1x[:, 0:n], sd[:, 0:n])
        for t in range(t0, t1):
            nc.vector.tensor_scalar(
                X[:, t, :].bitcast(F32R), X[:, t, :], mvs[:, t, 0:1],
                rstd1x[:, t - t0:t - t0 + 1],
                op0=ALU.subtract, op1=ALU.mult,
            )

    def emit_ln1(b, t0=0, t1=NT):
        mvs, e1 = emit_ln1_stats(b, t0, t1)
        emit_ln1_finish(b, mvs, e1, t0, t1)

    def emit_ht_alloc(b):
        S[b]["HT"] = htpool.tile([P, DC, N], F32R, tag="HT", name="HT")

    def emit_ht(b, t):
        """PE-transpose h chunk t -> HT feature-major (f32r, 1.5 cyc/row)."""
        s = S[b]
        pt = ps_t.tile([P, DC, P], F32R, tag="pst")
        for c in range(DC):
            nc.tensor.transpose(
                pt[:, c, :], s["X"][:, t, c * P:(c + 1) * P].bitcast(F32R),
                ident[:],
            )
        nc.vector.tensor_copy(s["HT"][:, :, t * P:(t + 1) * P], pt[:])

    def emit_v_alloc(b):
        s = S[b]
        s["VM"] = vmpool.tile([P, NT, D], BF16, tag="VM", name="VM")
        s["vmsum"] = vmpool.tile([P, NT], F32, tag="vmsum", name="vmsum")
        s["RS"] = vmpool.tile([P, NT, 2], BF16, tag="RS", name="RS")

    def emit_v(b, t):
        """V chunk t: matmul + masked copyback with free vmsum accumulation.
        Copybacks alternate ACT/DVE so neither engine's queue serializes the
        PSUM bank drain behind gelus or HT copies."""
        s = S[b]
        pm = ps_mm.tile([P, 512], F32, tag="psmm")
        for dc in range(DC):
            nc.tensor.matmul(
                pm[:], s["HT"][:, dc, t * P:(t + 1) * P], W["wv"][:, dc, :],
                start=(dc == 0), stop=(dc == DC - 1),
            )
        if t % 2 == 0:
            nc.scalar.activation(
                s["VM"][:, t, :], pm[:], ACT.Copy,
                scale=s["mask_f"][:, t:t + 1],
                accum_out=s["vmsum"][:, t:t + 1],
            )
        else:
            nc.vector.tensor_scalar(
                s["VM"][:, t, :], pm[:], s["mask_f"][:, t:t + 1], 0.0,
                op0=ALU.mult, op1=ALU.add, accum_out=s["vmsum"][:, t:t + 1],
            )

    def emit_rs(b):
        """RS stationary [P, m, 2]: col0 = vmsum/D (-> mu row), col1 =
        sqrt(eps)*mask (-> sqrt(eps)*r row)."""
        s = S[b]
        m = ms[b]
        nc.vector.tensor_scalar(
            s["RS"][:, 0:m, 0:1], s["vmsum"][:, 0:m].unsqueeze(-1), 1.0 / D,
            None, op0=ALU.mult,
        )
        nc.vector.tensor_scalar(
            s["RS"][:, 0:m, 1:2], s["mask_f"][:, 0:m].unsqueeze(-1), SQEPS,
            None, op0=ALU.mult,
        )

    def emit_qmt_alloc(b):
        S[b]["QMT"] = qmpool.tile([P, DC, N], F32R, tag="QMT", name="QMT")

    def emit_qmt(b, w2):
        """QM^T = M^T @ h^T, feature-major over token window w2."""
        s = S[b]
        t0 = w2 * W_
        for c in range(DC):
            pm = ps_mm.tile([P, 512], F32, tag="psmm")
            for dc in range(DC):
                nc.tensor.matmul(
                    pm[:], W["m"][:, dc, c * P:(c + 1) * P],
                    s["HT"][:, dc, t0:t0 + W_],
                    start=(dc == 0), stop=(dc == DC - 1),
                )
            nc.scalar.activation(s["QMT"][:, c, t0:t0 + W_], pm[:], ACT.Copy)

    def emit_scores(b, w):
        """scores^T chunks jc < m for query window w, exp -> PT (bf16)."""
        s = S[b]
        m = ms[b]
        q0 = w * W_
        PT = ptpool.tile([P, NT, W_], BF16, tag="PT", name="PT")
        s[f"PT{w}"] = PT
        for jc in range(m):
            pm = ps_mm.tile([P, 512], F32, tag="psmm")
            for dc in range(DC):
                nc.tensor.matmul(
                    pm[:], s["HT"][:, dc, jc * P:(jc + 1) * P],
                    s["QMT"][:, dc, q0:q0 + W_],
                    start=(dc == 0), stop=(dc == DC - 1),
                )
            nc.scalar.activation(PT[:, jc, :], pm[:], ACT.Exp)

    def emit_y(b, w):
        """Y^T = VM^T @ PT (feature-major) + rows (r/mu via RS, ssq via ones).

        Raw copyback on Pool (frees the bank fast), Square on ACT from PSUM.
        """
        s = S[b]
        m = ms[b]
        PT = s[f"PT{w}"]
        YT = ytpool.tile([P, DC, W_], F32R, tag="YT", name="YT")
        YSQ = ysqpool.tile([P, DC, W_], F32R, tag="YSQ", name="YSQ")
        s[f"YT{w}"] = YT
        # rows first: pr2 = [mu; sqrt(eps)*r], its square runs on DVE while
        # the y matmuls stream, so the e-matmul below never stalls the PE
        pr2 = ps_r.tile([2, W_], F32, tag="pr2", name="pr2")
        for jc in range(m):
            nc.tensor.matmul(
                pr2[:], s["RS"][:, jc, :], PT[:, jc, :],
                start=(jc == 0), stop=(jc == m - 1),
            )
        sq2 = rows.tile([2, W_], F32R, tag="sq2", name="sq2")
        nc.scalar.activation(sq2[:], pr2[:], ACT.Square)
        for fc in range(DC):
            pm = ps_mm.tile([P, 512], F32, tag="psmm")
            for jc in range(m):
                nc.tensor.matmul(
                    pm[:], s["VM"][:, jc, fc * P:(fc + 1) * P], PT[:, jc, :],
                    start=(jc == 0), stop=(jc == m - 1),
                )
            nc.scalar.activation(YSQ[:, fc, :], pm[:], ACT.Square)
            nc.scalar.activation(YT[:, fc, :], pm[:], ACT.Copy)
        # e = ssq/D - mu^2 + eps*r^2 accumulated fully on the PE
        ps1 = ps_r.tile([1, W_], F32, tag="ps1", name="ps1")
        for fc in range(DC):
            nc.tensor.matmul(
                ps1[:], onesD[:, 0:1].bitcast(F32R), YSQ[:, fc, :],
                start=(fc == 0), stop=False,
            )
        nc.tensor.matmul(ps1[:], pm1su[:], sq2[:], start=False, stop=True)
        muneg = rows.tile([1, W_], F32, tag="muneg", name="muneg", bufs=2)
        nc.vector.tensor_scalar(muneg[:], pr2[0:1, :], -1.0, None, op0=ALU.mult)
        MU = rfull.tile([P, W_], F32, tag="MU", name="MU")
        nc.gpsimd.partition_broadcast(MU[:], muneg[:])
        s[f"MU{w}"] = MU
        s[f"ps1_{w}"] = ps1

    def gate(dst_ap, gate_ap):
        """1-element no-op write that adds an artificial dependency on
        gate_ap, keeping a table-switching ACT consumer of dst from being
        greedily scheduled into an earlier table period."""
        nc.vector.scalar_tensor_tensor(
            dst_ap, dst_ap, 1.0, gate_ap, op0=ALU.mult, op1=ALU.bypass,
        )

    def emit_rstd(b, w, gate_ap=None):
        """rstd row = 1/sqrt(e): fast reciprocal on DVE + Sqrt on ACT (the
        windows' Sqrts are gated to sit adjacent, one sqrt-table period),
        broadcast on Pool."""
        s = S[b]
        ps1 = s[f"ps1_{w}"]
        rec = rows.tile([1, W_], F32, tag="rec", name="rec", bufs=2)
        nc.vector.reciprocal_approx_fast(rec[:], ps1[:])
        if gate_ap is not None:
            gate(rec[0:1, 0:1], gate_ap)
        rstd = rows.tile([1, W_], F32, tag="rstd", name="rstd", bufs=2)
        nc.scalar.activation(rstd[:], rec[:], ACT.Sqrt)
        RF = rfull.tile([P, W_], F32, tag="RF", name="RF")
        nc.gpsimd.partition_broadcast(RF[:], rstd[:])
        s[f"RF{w}"] = RF
        s[f"rec{w}"] = rec

    def emit_fc(b, w):
        """fc on the UNSCALED Y (the rstd column scale commutes through the
        contraction). The rank-1 mean correction and the rstd scale are both
        applied on DVE between PSUM and gelu:
        zs = (fc_raw + (-mu) * wfsum_col) * rstd."""
        s = S[b]
        YT, RF, MU = s[f"YT{w}"], s[f"RF{w}"], s[f"MU{w}"]
        ZT = ztpool.tile([P, DC, W_], F32R, tag="ZT", name="ZT", bufs=2)
        s[f"ZT{w}"] = ZT
        for c in range(DC):
            pm = ps_mm.tile([P, 512], F32, tag="psmm")
            for ec in range(DC):
                nc.tensor.matmul(
                    pm[:], W["wf"][:, ec, c * P:(c + 1) * P], YT[:, ec, :],
                    start=(ec == 0), stop=(ec == DC - 1),
                )
            zs = zpool.tile([P, W_], F32, tag="zs", name="zs")
            nc.vector.scalar_tensor_tensor(
                zs[:], MU[:], wfsp[:, c:c + 1], pm[:],
                op0=ALU.mult, op1=ALU.add,
            )
            nc.vector.tensor_tensor(zs[:], zs[:], RF[:], ALU.mult)
            nc.scalar.activation(ZT[:, c, :], zs[:], ACT.Gelu)

    def emit_proj(b, w):
        """proj token-major + copyback + store."""
        s = S[b]
        ZT = s[f"ZT{w}"]
        ob = out[b].rearrange("(t p) d -> p t d", p=P)
        for tc in range(DC):
            pm = ps_mm.tile([P, 512], F32, tag="psmm")
            for c in range(DC):
                nc.tensor.matmul(
                    pm[:], ZT[:, c, tc * P:(tc + 1) * P], W["wp"][:, c, :],
                    start=(c == 0), stop=(c == DC - 1),
                )
            o = outp.tile([P, D], F32, tag="O")
            nc.vector.tensor_copy(o[:], pm[:])
            nc.sync.dma_start(ob[:, w * DC + tc, :], o[:])

    def warm_burst(k0, n_mm, last):
        wpm = ps_mm.tile([P, 512], F32, tag="psmm", name="warmmm")
        for k in range(n_mm):
            nc.tensor.matmul(
                wpm[:], W["wv"][:, (k0 + k) % DC, 0:P],
                W["wv"][:, (k0 + k) % DC, :],
                start=(k == 0), stop=(k == n_mm - 1),
            )
        if last:
            wsb = outp.tile([P, 8], F32, tag="O", name="warmsb")
            nc.vector.tensor_copy(wsb[:], wpm[:, 0:8])
            nc.sync.dma_start(warm[:], wsb[:])

    # ---- startup: slot 0's A/B/C phases with PE warm-up interleaved ----
    load_w("wv")
    load_w("m")
    emit_xdma(0, split=True)
    warm_burst(0, 10, False)
    load_w("wf")
    load_w("wp")
    emit_ln1(0, 0, 2)
    emit_ht_alloc(0)
    emit_v_alloc(0)
    warm_burst(10, 8, False)
    emit_ht(0, 0)
    emit_ln1(0, 2, 4)
    warm_burst(18, 8, False)
    emit_ht(0, 1)
    if ms[0] > 0:
        emit_v(0, 0)
    warm_burst(26, 8, False)
    emit_ht(0, 2)
    emit_ln1(0, 4, 6)
    if ms[0] > 1:
        emit_v(0, 1)
    warm_burst(34, 4, True)
    emit_ht(0, 3)
    if ms[0] > 2:
        emit_v(0, 2)
    emit_ln1(0, 6, 8)
    if ms[0] > 3:
        emit_v(0, 3)
    for t in range(4, NT):
        emit_ht(0, t)
        if t < ms[0]:
            emit_v(0, t)
    emit_rs(0)
    emit_qmt_alloc(0)
    emit_qmt(0, 0)
    emit_qmt(0, 1)

    # ---- steady-state slot pipeline ----
    for b in range(G):
        nb = b + 1 if b + 1 < G else None
        emit_scores(b, 0)
        if nb is not None:
            emit_xdma(nb)
        emit_y(b, 0)
        emit_scores(b, 1)
        if nb is not None:
            ln1s = emit_ln1_stats(nb)   # DVE work overlapping PE scores
        emit_y(b, 1)
        # gate rows0's Sqrt behind the last window-1 exp so the greedy ACT
        # scheduler cannot pull it into the exp-table period
        emit_rstd(b, 0, gate_ap=S[b]["PT1"][0:1, ms[b] - 1, 0:2].bitcast(F32))
        emit_rstd(b, 1)
        if nb is not None:
            # LN1 Sqrt joins the sqrt-table period
            emit_ln1_finish(nb, *ln1s, gate_ap=S[b]["rec1"][0:1, 0:1])
        emit_fc(b, 0)
        emit_fc(b, 1)           # gelus adjacent on ACT (one table period)
        if nb is not None:
            emit_ht_alloc(nb)
            emit_v_alloc(nb)
            for t in range(NT):
                emit_ht(nb, t)
                if t < ms[nb]:
                    emit_v(nb, t)
            emit_rs(nb)
        emit_proj(b, 0)
        if nb is not None:
            emit_qmt_alloc(nb)
            emit_qmt(nb, 0)
            emit_qmt(nb, 1)
        emit_proj(b, 1)


def build(budgets):
    from contextlib import ExitStack

    nc = bacc.Bacc("TRN2", target_bir_lowering=False, debug=False,
                   num_devices=NCORES)
    x = nc.dram_tensor("x", [G, N, D], F32, kind="ExternalInput").ap()
    msk = nc.dram_tensor("msk", [G, N], F32, kind="ExternalInput").ap()
    wdram = {
        name: nc.dram_tensor(name, [D, D], F32, kind="ExternalInput").ap()
        for name in ("m", "wv", "wf", "wp")
    }
    wdram["wfsp"] = nc.dram_tensor(
        "wfsp", [P, DC], F32, kind="ExternalInput"
    ).ap()
    wdram["pm1su"] = nc.dram_tensor(
        "pm1su", [2, 1], F32, kind="ExternalInput"
    ).ap()
    wdram["onesd"] = nc.dram_tensor(
        "onesd", [P, 1], F32, kind="ExternalInput"
    ).ap()
    wdram["identr"] = nc.dram_tensor(
        "identr", [P, P], F32, kind="ExternalInput"
    ).ap()
    out = nc.dram_tensor("out", [G, N, D], F32, kind="ExternalOutput").ap()
    warm = nc.dram_tensor("warm", [P, 8], F32, kind="ExternalOutput").ap()

    with tile.TileContext(nc) as tc:
        with ExitStack() as ctx:
            _body(ctx, tc, x, msk, wdram, out, warm, budgets)
    nc.compile()
    return nc


_NC_CACHE = {}


def get_nc(budgets):
    budgets = tuple(int(m) for m in budgets)
    if budgets not in _NC_CACHE:
        _NC_CACHE[budgets] = build(budgets)
    return _NC_CACHE[budgets]


def plan(belief_base_sizes):
    """Slot assignment: sort batches by chunk count desc, deal rank 8i+c to
    core c slot i. Budget m_i = chunk count of rank 8i (the group max)."""
    sizes = np.asarray(belief_base_sizes, dtype=np.int64)
    chunks = -(-sizes // P)
    order = np.argsort(-chunks, kind="stable")
    budgets = tuple(int(chunks[order[NCORES * i]]) for i in range(G))
    # batch_for[c][i] = original batch index handled by core c, slot i
    batch_for = [[int(order[NCORES * i + c]) for i in range(G)]
                 for c in range(NCORES)]
    return budgets, batch_for


def make_in_maps(x, belief_base_sizes, g1, w_qkv, g2, w_fc, w_proj):
    x = np.asarray(x, dtype=np.float32)
    sizes = np.asarray(belief_base_sizes, dtype=np.int64)
    g1 = np.asarray(g1, dtype=np.float32)
    w_qkv = np.asarray(w_qkv, dtype=np.float32)
    g2 = np.asarray(g2, dtype=np.float32)
    w_fc = np.asarray(w_fc, dtype=np.float32)
    w_proj = np.asarray(w_proj, dtype=np.float32)

    wq = (g1[:, None] * w_qkv[:, :D]) / np.float32(np.sqrt(D))
    wk = g1[:, None] * w_qkv[:, D:2 * D]
    m_mat = np.ascontiguousarray((wq @ wk.T).astype(np.float32))
    wv = np.ascontiguousarray(g1[:, None] * w_qkv[:, 2 * D:])
    wf = np.ascontiguousarray(g2[:, None] * w_fc)
    wfsp = np.ascontiguousarray(wf.sum(0).reshape(DC, P).T.copy())
    pm1su = np.array([[-1.0], [1.0]], dtype=np.float32)
    onesd = np.full((P, 1), 1.0 / D, dtype=np.float32)
    identr = np.eye(P, dtype=np.float32)
    wp = np.ascontiguousarray(w_proj)

    mask = (np.arange(N)[None, :] < sizes[:, None]).astype(np.float32)

    budgets, batch_for = plan(sizes)
    in_maps = []
    for c in range(NCORES):
        sel = batch_for[c]
        in_maps.append({
            "x": np.ascontiguousarray(x[sel]),
            "msk": np.ascontiguousarray(mask[sel]),
            "m": m_mat, "wv": wv, "wf": wf, "wp": wp, "wfsp": wfsp,
            "pm1su": pm1su, "onesd": onesd, "identr": identr,
        })
    return budgets, batch_for, in_maps


def kernel(x, belief_base_sizes, g1, w_qkv, g2, w_fc, w_proj):
    budgets, batch_for, in_maps = make_in_maps(
        x, belief_base_sizes, g1, w_qkv, g2, w_fc, w_proj
    )
    nc = get_nc(budgets)
    res = run_bass_kernel_spmd(nc, in_maps, core_ids=list(range(NCORES)))
    out = np.empty((B, N, D), dtype=np.float32)
    for c in range(NCORES):
        oc = res.results[c]["out"]
        for i in range(G):
            out[batch_for[c][i]] = oc[i]
    return np.ascontiguousarray(out)


# revision 24
# speedup vs baseline: 1.1593x; 1.1593x over previous
"""Trainium2 Bass kernel for a belief-transformer block (sparse attention).

Computation (per batch b):
    h   = LayerNorm(x[b]) * g1
    qkv = h @ w_qkv ; q,k,v = split(qkv)
    s   = q @ k^T / sqrt(D), keys j >= L_b masked
    y   = softmax(s) @ v
    y   = LayerNorm(y) * g2
    out = gelu(y @ w_fc) @ w_proj

Key optimizations over a dense data-parallel implementation:

* Sparsity: keys >= L_b contribute nothing (masked to -inf before softmax),
  so scores / attn@v / k,v-projection are computed only for the first
  ceil(L_b/128) key chunks. The SPMD program is JIT-specialized to per-slot
  chunk budgets: batches are sorted by chunk count and dealt round-robin so
  slot i's budget is the max chunk count of rank group [8i, 8i+8) -- the
  provably minimal total budget for an identical-program 8-core layout.
* M-trick: scores = (h @ M) @ h^T with M = (g1*wq/sqrt(D)) @ (g1*wk)^T folded
  on the host -- the entire K projection disappears.
* Feature-major attention output Y^T = V^T @ P^T: no transposes between
  attention and the MLP. LN2 statistics come out as [1, 512] rows: the mean
  via a vmsum column in the same 2-row matmul that computes the softmax
  denominator row r (with sqrt(eps) folded so eps*r^2 = row^2), sum-of-squares
  via a ones-column matmul over YSQ. rstd = Exp(-0.5*Ln(e)) on the ACT engine
  keeps everything inside the ln/exp activation table (no table switches).
  LN2 normalization is folded into the fc matmul: YT is pre-scaled by the
  rstd row and the mean term becomes a rank-1 correction row ([-colsum(wf)]
  stationary x (mu*rstd) moving) appended to the accumulation group.
* Attention weights (PT), masked values (VM) and the stats stationary (RS)
  are bf16: full-rate matmuls, half the SBUF traffic; error ~2.7e-3 vs the
  2e-2 gate.
* Engine balancing: PSUM->SBUF copybacks on the otherwise idle GpSimd/Pool
  engine, LN applies + row math on DVE, exp/gelu/squares/V-masking on ACT
  (Copy/Square live in every activation table), PE does only matmuls and
  128-wide f32r transposes (1.5 cyc/row). PE warm-up bursts at startup keep
  the p-state clock ramped.
"""

import os
import sys

import numpy as np


def _ensure_concourse():
    try:
        import concourse  # noqa: F401
        return
    except ImportError:
        pass
    for p in ("/root/.axon_site/_ro/trn_rl_repo", "/opt/trn_rl_repo"):
        if os.path.isdir(p) and p not in sys.path:
            sys.path.insert(0, p)
    import concourse  # noqa: F401


_ensure_concourse()

import concourse.tile as tile  # noqa: E402
from concourse import bacc, mybir  # noqa: E402
from concourse.bass_utils import run_bass_kernel_spmd  # noqa: E402
from concourse.masks import make_identity  # noqa: E402

B, N, D = 32, 1024, 512
NCORES = 8
G = B // NCORES  # batch slots per core
P = 128
NT = N // P      # token chunks (8)
DC = D // P      # feature chunks (4)
W_ = 512         # query window (2 windows per batch)
EPS = 1e-5
SQEPS = float(np.sqrt(np.float32(EPS)))

F32 = mybir.dt.float32
F32R = mybir.dt.float32r
BF16 = mybir.dt.bfloat16
ALU = mybir.AluOpType
ACT = mybir.ActivationFunctionType


def _body(ctx, tc, x, msk, wdram, out, warm, ms):
    nc = tc.nc

    singles = ctx.enter_context(tc.tile_pool(name="singles", bufs=1))
    xpool = ctx.enter_context(tc.tile_pool(name="xpool", bufs=2))
    htpool = ctx.enter_context(tc.tile_pool(name="htpool", bufs=2))
    qmpool = ctx.enter_context(tc.tile_pool(name="qmpool", bufs=1))
    vmpool = ctx.enter_context(tc.tile_pool(name="vmpool", bufs=1))
    ptpool = ctx.enter_context(tc.tile_pool(name="ptpool", bufs=2))
    ytpool = ctx.enter_context(tc.tile_pool(name="ytpool", bufs=2))
    ysqpool = ctx.enter_context(tc.tile_pool(name="ysqpool", bufs=1))
    ztpool = ctx.enter_context(tc.tile_pool(name="ztpool", bufs=1))
    rows = ctx.enter_context(tc.tile_pool(name="rows", bufs=1))
    zpool = ctx.enter_context(tc.tile_pool(name="zpool", bufs=2))
    rfull = ctx.enter_context(tc.tile_pool(name="rfull", bufs=2))
    stats = ctx.enter_context(tc.tile_pool(name="stats", bufs=2))
    outp = ctx.enter_context(tc.tile_pool(name="outp", bufs=2))
    ps_mm = ctx.enter_context(tc.tile_pool(name="ps_mm", bufs=4, space="PSUM"))
    ps_t = ctx.enter_context(tc.tile_pool(name="ps_t", bufs=2, space="PSUM"))
    ps_r = ctx.enter_context(tc.tile_pool(name="ps_r", bufs=1, space="PSUM"))

    # Replicated weights, feature-chunked [P, DC, D]; f32r for the PE.
    W = {}

    def load_w(name):
        t = singles.tile([P, DC, D], F32R, tag=name, name=name)
        nc.sync.dma_start(
            t[:], wdram[name].rearrange("(c p) e -> p c e", p=P).bitcast(F32R)
        )
        W[name] = t

    ident = singles.tile([P, P], F32R, tag="ident", name="ident")
    nc.sync.dma_start(ident[:], wdram["identr"].bitcast(F32R))
    onesD = singles.tile([P, 1], F32R, tag="onesD", name="onesD")
    nc.sync.dma_start(onesD[:], wdram["onesd"].bitcast(F32R))
    wfsp = singles.tile([P, DC], F32, tag="wfsp", name="wfsp")
    nc.sync.dma_start(wfsp[:], wdram["wfsp"])
    pm1su_t = singles.tile([2, 1], F32R, tag="pm1su", name="pm1su_t")
    nc.sync.dma_start(pm1su_t[:], wdram["pm1su"].bitcast(F32R))
    pm1su = pm1su_t[:]

    S = [dict() for _ in range(G)]  # per-slot live tiles

    def emit_xdma(b, split=False):
        s = S[b]
        xb = x[b].rearrange("(t p) d -> p t d", p=P)
        mb = msk[b].rearrange("(t p) -> p t", p=P)
        X = xpool.tile([P, NT, D], F32, tag="X", name="X")
        s["X"] = X
        if split:
            for pc in range(4):
                nc.sync.dma_start(
                    X[:, 2 * pc:2 * pc + 2, :].bitcast(F32R),
                    xb[:, 2 * pc:2 * pc + 2, :].bitcast(F32R),
                )
        else:
            nc.sync.dma_start(X[:].bitcast(F32R), xb.bitcast(F32R))
        s["mask_f"] = xpool.tile([P, NT], F32, tag="mask_f", name="mask_f")
        nc.sync.dma_start(s["mask_f"][:], mb)

    def emit_ln1_stats(b, t0=0, t1=NT):
        """LN1 stats for X chunks [t0, t1): bn stats + sd = Sqrt(var+eps) on
        ACT (one [P, n] op -- emit next to the other Sqrt users so they share
        the sqrt-table period)."""
        X = S[b]["X"]
        n = t1 - t0
        mvs = stats.tile([P, NT, 2], F32, tag="bnag", name="mvs", bufs=3)
        for t in range(t0, t1):
            st = stats.tile([P, 6], F32, tag="bnst")
            nc.vector.bn_stats(st[:], X[:, t, :])
            nc.vector.bn_aggr(mvs[:, t, :], st[:])
        e1 = stats.tile([P, NT], F32, tag="ln1e", name="e1", bufs=3)
        nc.vector.tensor_scalar(
            e1[:, 0:n], mvs[:, t0:t1, 1], EPS, None, op0=ALU.add
        )
        return mvs, e1

    def emit_ln1_finish(b, mvs, e1, t0=0, t1=NT, gate_ap=None):
        """Sqrt (ACT, cluster with other Sqrt users) + fast reciprocal, then
        the normalize applies on DVE."""
        X = S[b]["X"]
        n = t1 - t0
        if gate_ap is not None:
            gate(e1[0:1, 0:1], gate_ap)
        sd = stats.tile([P, NT], F32, tag="ln1sd", name="sd", bufs=3)
        nc.scalar.activation(sd[:, 0:n], e1[:, 0:n], ACT.Sqrt)
        rstd1x = stats.tile([P, NT], F32, tag="rstd1x", name="rstd1x", bufs=3)
        nc.vector.reciprocal_approx_fast(rstd1x[:, 0:n], sd[:, 0:n])
        for t in range(t0, t1):
            nc.vector.tensor_scalar(
                X[:, t, :].bitcast(F32R), X[:, t, :], mvs[:, t, 0:1],
                rstd1x[:, t - t0:t - t0 + 1],
                op0=ALU.subtract, op1=ALU.mult,
            )

    def emit_ln1(b, t0=0, t1=NT):
        mvs, e1 = emit_ln1_stats(b, t0, t1)
        emit_ln1_finish(b, mvs, e1, t0, t1)

    def emit_ht_alloc(b):
        S[b]["HT"] = htpool.tile([P, DC, N], F32R, tag="HT", name="HT")

    def emit_ht(b, t):
        """PE-transpose h chunk t -> HT feature-major (f32r, 1.5 cyc/row)."""
        s = S[b]
        pt = ps_t.tile([P, DC, P], F32R, tag="pst")
        for c in range(DC):
            nc.tensor.transpose(
                pt[:, c, :], s["X"][:, t, c * P:(c + 1) * P].bitcast(F32R),
                ident[:],
            )
        nc.vector.tensor_copy(s["HT"][:, :, t * P:(t + 1) * P], pt[:])

    def emit_v_alloc(b):
        s = S[b]
        s["VM"] = vmpool.tile([P, NT, D], BF16, tag="VM", name="VM")
        s["vmsum"] = vmpool.tile([P, NT], F32, tag="vmsum", name="vmsum")
        s["RS"] = vmpool.tile([P, NT, 2], BF16, tag="RS", name="RS")

    def emit_v(b, t):
        """V chunk t: matmul + masked copyback with free vmsum accumulation.
        Copybacks alternate ACT/DVE so neither engine's queue serializes the
        PSUM bank drain behind gelus or HT copies."""
        s = S[b]
        pm = ps_mm.tile([P, 512], F32, tag="psmm")
        for dc in range(DC):
            nc.tensor.matmul(
                pm[:], s["HT"][:, dc, t * P:(t + 1) * P], W["wv"][:, dc, :],
                start=(dc == 0), stop=(dc == DC - 1),
            )
        if t % 2 == 0:
            nc.scalar.activation(
                s["VM"][:, t, :], pm[:], ACT.Copy,
                scale=s["mask_f"][:, t:t + 1],
                accum_out=s["vmsum"][:, t:t + 1],
            )
        else:
            nc.vector.tensor_scalar(
                s["VM"][:, t, :], pm[:], s["mask_f"][:, t:t + 1], 0.0,
                op0=ALU.mult, op1=ALU.add, accum_out=s["vmsum"][:, t:t + 1],
            )

    def emit_rs(b):
        """RS stationary [P, m, 2]: col0 = vmsum/D (-> mu row), col1 =
        sqrt(eps)*mask (-> sqrt(eps)*r row)."""
        s = S[b]
        m = ms[b]
        nc.vector.tensor_scalar(
            s["RS"][:, 0:m, 0:1], s["vmsum"][:, 0:m].unsqueeze(-1), 1.0 / D,
            None, op0=ALU.mult,
        )
        nc.vector.tensor_scalar(
            s["RS"][:, 0:m, 1:2], s["mask_f"][:, 0:m].unsqueeze(-1), SQEPS,
            None, op0=ALU.mult,
        )

    def emit_qmt_alloc(b):
        S[b]["QMT"] = qmpool.tile([P, DC, N], F32R, tag="QMT", name="QMT")

    def emit_qmt(b, w2):
        """QM^T = M^T @ h^T, feature-major over token window w2."""
        s = S[b]
        t0 = w2 * W_
        for c in range(DC):
            pm = ps_mm.tile([P, 512], F32, tag="psmm")
            for dc in range(DC):
                nc.tensor.matmul(
                    pm[:], W["m"][:, dc, c * P:(c + 1) * P],
                    s["HT"][:, dc, t0:t0 + W_],
                    start=(dc == 0), stop=(dc == DC - 1),
                )
            nc.scalar.activation(s["QMT"][:, c, t0:t0 + W_], pm[:], ACT.Copy)

    def emit_scores(b, w):
        """scores^T chunks jc < m for query window w, exp -> PT (bf16)."""
        s = S[b]
        m = ms[b]
        q0 = w * W_
        PT = ptpool.tile([P, NT, W_], BF16, tag="PT", name="PT")
        s[f"PT{w}"] = PT
        for jc in range(m):
            pm = ps_mm.tile([P, 512], F32, tag="psmm")
            for dc in range(DC):
                nc.tensor.matmul(
                    pm[:], s["HT"][:, dc, jc * P:(jc + 1) * P],
                    s["QMT"][:, dc, q0:q0 + W_],
                    start=(dc == 0), stop=(dc == DC - 1),
                )
            nc.scalar.activation(PT[:, jc, :], pm[:], ACT.Exp)

    def emit_y(b, w):
        """Y^T = VM^T @ PT (feature-major) + rows (r/mu via RS, ssq via ones).

        Raw copyback on Pool (frees the bank fast), Square on ACT from PSUM.
        """
        s = S[b]
        m = ms[b]
        PT = s[f"PT{w}"]
        YT = ytpool.tile([P, DC, W_], F32R, tag="YT", name="YT")
        YSQ = ysqpool.tile([P, DC, W_], F32R, tag="YSQ", name="YSQ")
        s[f"YT{w}"] = YT
        # rows first: pr2 = [mu; sqrt(eps)*r], its square runs on DVE while
        # the y matmuls stream, so the e-matmul below never stalls the PE
        pr2 = ps_r.tile([2, W_], F32, tag="pr2", name="pr2")
        for jc in range(m):
            nc.tensor.matmul(
                pr2[:], s["RS"][:, jc, :], PT[:, jc, :],
                start=(jc == 0), stop=(jc == m - 1),
            )
        sq2 = rows.tile([2, W_], F32R, tag="sq2", name="sq2")
        nc.scalar.activation(sq2[:], pr2[:], ACT.Square)
        for fc in range(DC):
            pm = ps_mm.tile([P, 512], F32, tag="psmm")
            for jc in range(m):
                nc.tensor.matmul(
                    pm[:], s["VM"][:, jc, fc * P:(fc + 1) * P], PT[:, jc, :],
                    start=(jc == 0), stop=(jc == m - 1),
                )
            nc.scalar.activation(YSQ[:, fc, :], pm[:], ACT.Square)
            nc.scalar.activation(YT[:, fc, :], pm[:], ACT.Copy)
        # e = ssq/D - mu^2 + eps*r^2 accumulated fully on the PE
        ps1 = ps_r.tile([1, W_], F32, tag="ps1", name="ps1")
        for fc in range(DC):
            nc.tensor.matmul(
                ps1[:], onesD[:, 0:1].bitcast(F32R), YSQ[:, fc, :],
                start=(fc == 0), stop=False,
            )
        nc.tensor.matmul(ps1[:], pm1su[:], sq2[:], start=False, stop=True)
        muneg = rows.tile([1, W_], F32, tag="muneg", name="muneg", bufs=2)
        nc.vector.tensor_scalar(muneg[:], pr2[0:1, :], -1.0, None, op0=ALU.mult)
        MU = rfull.tile([P, W_], F32, tag="MU", name="MU")
        nc.gpsimd.partition_broadcast(MU[:], muneg[:])
        s[f"MU{w}"] = MU
        s[f"ps1_{w}"] = ps1

    def gate(dst_ap, gate_ap):
        """1-element no-op write that adds an artificial dependency on
        gate_ap, keeping a table-switching ACT consumer of dst from being
        greedily scheduled into an earlier table period."""
        nc.vector.scalar_tensor_tensor(
            dst_ap, dst_ap, 1.0, gate_ap, op0=ALU.mult, op1=ALU.bypass,
        )

    def emit_rstd(b, w, gate_ap=None):
        """rstd row = 1/sqrt(e): fast reciprocal on DVE + Sqrt on ACT (the
        windows' Sqrts are gated to sit adjacent, one sqrt-table period),
        broadcast on Pool."""
        s = S[b]
        ps1 = s[f"ps1_{w}"]
        rec = rows.tile([1, W_], F32, tag="rec", name="rec", bufs=2)
        nc.vector.reciprocal_approx_fast(rec[:], ps1[:])
        if gate_ap is not None:
            gate(rec[0:1, 0:1], gate_ap)
        rstd = rows.tile([1, W_], F32, tag="rstd", name="rstd", bufs=2)
        nc.scalar.activation(rstd[:], rec[:], ACT.Sqrt)
        RF = rfull.tile([P, W_], F32, tag="RF", name="RF")
        nc.gpsimd.partition_broadcast(RF[:], rstd[:])
        s[f"RF{w}"] = RF
        s[f"rec{w}"] = rec

    def emit_fc(b, w):
        """fc on the UNSCALED Y (the rstd column scale commutes through the
        contraction). The rank-1 mean correction and the rstd scale are both
        applied on DVE between PSUM and gelu:
        zs = (fc_raw + (-mu) * wfsum_col) * rstd."""
        s = S[b]
        YT, RF, MU = s[f"YT{w}"], s[f"RF{w}"], s[f"MU{w}"]
        ZT = ztpool.tile([P, DC, W_], F32R, tag="ZT", name="ZT", bufs=2)
        s[f"ZT{w}"] = ZT
        for c in range(DC):
            pm = ps_mm.tile([P, 512], F32, tag="psmm")
            for ec in range(DC):
                nc.tensor.matmul(
                    pm[:], W["wf"][:, ec, c * P:(c + 1) * P], YT[:, ec, :],
                    start=(ec == 0), stop=(ec == DC - 1),
                )
            zs = zpool.tile([P, W_], F32, tag="zs", name="zs")
            nc.vector.scalar_tensor_tensor(
                zs[:], MU[:], wfsp[:, c:c + 1], pm[:],
                op0=ALU.mult, op1=ALU.add,
            )
            nc.vector.tensor_tensor(zs[:], zs[:], RF[:], ALU.mult)
            nc.scalar.activation(ZT[:, c, :], zs[:], ACT.Gelu)

    def emit_proj(b, w):
        """proj token-major + copyback + store."""
        s = S[b]
        ZT = s[f"ZT{w}"]
        ob = out[b].rearrange("(t p) d -> p t d", p=P)
        for tc in range(DC):
            pm = ps_mm.tile([P, 512], F32, tag="psmm")
            for c in range(DC):
                nc.tensor.matmul(
                    pm[:], ZT[:, c, tc * P:(tc + 1) * P], W["wp"][:, c, :],
                    start=(c == 0), stop=(c == DC - 1),
                )
            o = outp.tile([P, D], F32, tag="O")
            nc.vector.tensor_copy(o[:], pm[:])
            nc.sync.dma_start(ob[:, w * DC + tc, :], o[:])

    def warm_burst(k0, n_mm, last):
        wpm = ps_mm.tile([P, 512], F32, tag="psmm", name="warmmm")
        for k in range(n_mm):
            nc.tensor.matmul(
                wpm[:], W["wv"][:, (k0 + k) % DC, 0:P],
                W["wv"][:, (k0 + k) % DC, :],
                start=(k == 0), stop=(k == n_mm - 1),
            )
        if last:
            wsb = outp.tile([P, 8], F32, tag="O", name="warmsb")
            nc.vector.tensor_copy(wsb[:], wpm[:, 0:8])
            nc.sync.dma_start(warm[:], wsb[:])

    # ---- startup: slot 0's A/B/C phases with PE warm-up interleaved ----
    load_w("wv")
    load_w("m")
    emit_xdma(0, split=True)
    warm_burst(0, 10, False)
    load_w("wf")
    load_w("wp")
    emit_ln1(0, 0, 2)
    emit_ht_alloc(0)
    emit_v_alloc(0)
    warm_burst(10, 8, False)
    emit_ht(0, 0)
    emit_ln1(0, 2, 4)
    warm_burst(18, 8, False)
    emit_ht(0, 1)
    if ms[0] > 0:
        emit_v(0, 0)
    warm_burst(26, 8, False)
    emit_ht(0, 2)
    emit_ln1(0, 4, 6)
    if ms[0] > 1:
        emit_v(0, 1)
    warm_burst(34, 4, True)
    emit_ht(0, 3)
    if ms[0] > 2:
        emit_v(0, 2)
    emit_ln1(0, 6, 8)
    if ms[0] > 3:
        emit_v(0, 3)
    for t in range(4, NT):
        emit_ht(0, t)
        if t < ms[0]:
            emit_v(0, t)
    emit_rs(0)
    emit_qmt_alloc(0)
    emit_qmt(0, 0)
    emit_qmt(0, 1)

    # ---- steady-state slot pipeline ----
    for b in range(G):
        nb = b + 1 if b + 1 < G else None
        emit_scores(b, 0)
        if nb is not None:
            emit_xdma(nb)
        emit_y(b, 0)
        emit_scores(b, 1)
        if nb is not None:
            ln1s = emit_ln1_stats(nb)   # DVE work overlapping PE scores
        emit_y(b, 1)
        # gate rows0's Sqrt behind the last window-1 exp so the greedy ACT
        # scheduler cannot pull it into the exp-table period
        emit_rstd(b, 0, gate_ap=S[b]["PT1"][0:1, ms[b] - 1, 0:2].bitcast(F32))
        emit_rstd(b, 1)
        if nb is not None:
            # LN1 Sqrt joins the sqrt-table period
            emit_ln1_finish(nb, *ln1s, gate_ap=S[b]["rec1"][0:1, 0:1])
        emit_fc(b, 0)           # covers the LN1-apply chain on DVE
        if nb is not None:
            emit_ht_alloc(nb)
            emit_v_alloc(nb)
            for t in range(NT):
                emit_ht(nb, t)
                if t < ms[nb]:
                    emit_v(nb, t)
            emit_rs(nb)
        emit_fc(b, 1)
        emit_proj(b, 0)
        if nb is not None:
            emit_qmt_alloc(nb)
            emit_qmt(nb, 0)
            emit_qmt(nb, 1)
        emit_proj(b, 1)


def build(budgets):
    from contextlib import ExitStack

    nc = bacc.Bacc("TRN2", target_bir_lowering=False, debug=False,
                   num_devices=NCORES)
    x = nc.dram_tensor("x", [G, N, D], F32, kind="ExternalInput").ap()
    msk = nc.dram_tensor("msk", [G, N], F32, kind="ExternalInput").ap()
    wdram = {
        name: nc.dram_tensor(name, [D, D], F32, kind="ExternalInput").ap()
        for name in ("m", "wv", "wf", "wp")
    }
    wdram["wfsp"] = nc.dram_tensor(
        "wfsp", [P, DC], F32, kind="ExternalInput"
    ).ap()
    wdram["pm1su"] = nc.dram_tensor(
        "pm1su", [2, 1], F32, kind="ExternalInput"
    ).ap()
    wdram["onesd"] = nc.dram_tensor(
        "onesd", [P, 1], F32, kind="ExternalInput"
    ).ap()
    wdram["identr"] = nc.dram_tensor(
        "identr", [P, P], F32, kind="ExternalInput"
    ).ap()
    out = nc.dram_tensor("out", [G, N, D], F32, kind="ExternalOutput").ap()
    warm = nc.dram_tensor("warm", [P, 8], F32, kind="ExternalOutput").ap()

    with tile.TileContext(nc) as tc:
        with ExitStack() as ctx:
            _body(ctx, tc, x, msk, wdram, out, warm, budgets)
    nc.compile()
    return nc


_NC_CACHE = {}


def get_nc(budgets):
    budgets = tuple(int(m) for m in budgets)
    if budgets not in _NC_CACHE:
        _NC_CACHE[budgets] = build(budgets)
    return _NC_CACHE[budgets]


def plan(belief_base_sizes):
    """Slot assignment: sort batches by chunk count desc, deal rank 8i+c to
    core c slot i. Budget m_i = chunk count of rank 8i (the group max)."""
    sizes = np.asarray(belief_base_sizes, dtype=np.int64)
    chunks = -(-sizes // P)
    order = np.argsort(-chunks, kind="stable")
    budgets = tuple(int(chunks[order[NCORES * i]]) for i in range(G))
    # batch_for[c][i] = original batch index handled by core c, slot i
    batch_for = [[int(order[NCORES * i + c]) for i in range(G)]
                 for c in range(NCORES)]
    return budgets, batch_for


def make_in_maps(x, belief_base_sizes, g1, w_qkv, g2, w_fc, w_proj):
    x = np.asarray(x, dtype=np.float32)
    sizes = np.asarray(belief_base_sizes, dtype=np.int64)
    g1 = np.asarray(g1, dtype=np.float32)
    w_qkv = np.asarray(w_qkv, dtype=np.float32)
    g2 = np.asarray(g2, dtype=np.float32)
    w_fc = np.asarray(w_fc, dtype=np.float32)
    w_proj = np.asarray(w_proj, dtype=np.float32)

    wq = (g1[:, None] * w_qkv[:, :D]) / np.float32(np.sqrt(D))
    wk = g1[:, None] * w_qkv[:, D:2 * D]
    m_mat = np.ascontiguousarray((wq @ wk.T).astype(np.float32))
    wv = np.ascontiguousarray(g1[:, None] * w_qkv[:, 2 * D:])
    wf = np.ascontiguousarray(g2[:, None] * w_fc)
    wfsp = np.ascontiguousarray(wf.sum(0).reshape(DC, P).T.copy())
    pm1su = np.array([[-1.0], [1.0]], dtype=np.float32)
    onesd = np.full((P, 1), 1.0 / D, dtype=np.float32)
    identr = np.eye(P, dtype=np.float32)
    wp = np.ascontiguousarray(w_proj)

    mask = (np.arange(N)[None, :] < sizes[:, None]).astype(np.float32)

    budgets, batch_for = plan(sizes)
    in_maps = []
    for c in range(NCORES):
        sel = batch_for[c]
        in_maps.append({
            "x": np.ascontiguousarray(x[sel]),
            "msk": np.ascontiguousarray(mask[sel]),
            "m": m_mat, "wv": wv, "wf": wf, "wp": wp, "wfsp": wfsp,
            "pm1su": pm1su, "onesd": onesd, "identr": identr,
        })
    return budgets, batch_for, in_maps


def kernel(x, belief_base_sizes, g1, w_qkv, g2, w_fc, w_proj):
    budgets, batch_for, in_maps = make_in_maps(
        x, belief_base_sizes, g1, w_qkv, g2, w_fc, w_proj
    )
    nc = get_nc(budgets)
    res = run_bass_kernel_spmd(nc, in_maps, core_ids=list(range(NCORES)))
    out = np.empty((B, N, D), dtype=np.float32)
    for c in range(NCORES):
        oc = res.results[c]["out"]
        for i in range(G):
            out[batch_for[c][i]] = oc[i]
    return np.ascontiguousarray(out)
